# revision 28
# baseline (speedup 1.0000x reference)
"""Trainium2 Bass kernel for CloudSSM: depthwise 5x5 conv + SiLU + selective-scan SSM.

Contract: kernel(**inputs) takes the FULL unsharded inputs (as in setup_inputs())
and returns the FULL [32, 96, 64, 64] float32 output. Internally shards the batch
(32) across 8 NeuronCores (4 per core), runs a Bass/Tile kernel via
run_bass_kernel_spmd, and gathers.

Self-contained: hardcodes all shapes; only imports the concourse runtime.
"""
import sys
import numpy as np
from contextlib import ExitStack

for _p in ("/opt/trn_rl_repo", "/root/.axon_site", "/root/.axon_site/_ro/trn_rl_repo"):
    if _p not in sys.path:
        sys.path.append(_p)

import concourse.bass as bass
import concourse.bacc as bacc
import concourse.tile as tile
from concourse import mybir
from concourse import bass_utils

AFT = mybir.ActivationFunctionType
ALU = mybir.AluOpType
F32 = mybir.dt.float32
F32R = mybir.dt.float32r
BF16 = mybir.dt.bfloat16

# problem shapes (hardcoded per spec)
B, D, H, W = 32, 96, 64, 64
N, R, KC = 4, 6, 5
L = H * W                      # 4096
NCORES = 8
BPC = B // NCORES              # 4 batches per core
HP, WP = H + 4, W + 4          # padded image 68x68
HALF = L // 2                  # 2048
CHUNK = 512
NCH = HALF // CHUNK            # 4 chunks per half
ROWS_PER_CHUNK = CHUNK // W    # 8 image rows per chunk
EPS = np.float32(1e-9)
KREC = float(np.float32(1.0) / EPS)
LNKREC = float(np.log(np.float32(KREC), dtype=np.float32))

CONV_F32R = True               # conv matmuls in fp32r (1 cyc/row) vs fp32 (4 cyc/row)
PROJ_F32R = True               # B/C broadcast matmuls in fp32r


def _mmdt(ap):
    """dtype view used for the fast matmuls"""
    return ap.bitcast(F32R)


def _patch_act_tables():
    """Force all activations onto the natural_log_exp_and_others table set
    (contains Copy/Exp/Identity/Ln/Relu - everything this kernel uses), so the
    whole kernel needs exactly one ACT_TABLE_LOAD instead of thrashing sets."""
    import concourse.hw_specs as hw_specs
    real = hw_specs.get_activation_tables

    def only_lnexp(module_arch):
        t = real(module_arch)
        return {k: (v if k == "natural_log_exp_and_others" else set())
                for k, v in t.items()}

    bacc.get_activation_tables = only_lnexp


def build_nc():
    _patch_act_tables()
    nc = bacc.Bacc("TRN2", target_bir_lowering=False, debug=False)

    xpad_d = nc.dram_tensor("xpad", [BPC, D, HP * WP], F32, kind="ExternalInput").ap()
    dw_d = nc.dram_tensor("dw", [D, 25 * D], F32, kind="ExternalInput").ap()
    mdt_d = nc.dram_tensor("mdt", [D, D], F32, kind="ExternalInput").ap()
    bw_d = nc.dram_tensor("bw", [D, N * D], F32, kind="ExternalInput").ap()
    cw_d = nc.dram_tensor("cw", [D, N * D], F32, kind="ExternalInput").ap()
    # small per-partition params packed in one tensor:
    # col 0: conv_b, col 1: dt_b, cols 2..5: A (=-exp(A_logs)), col 6: Ds
    prm_d = nc.dram_tensor("prm", [D, 16], F32, kind="ExternalInput").ap()
    y_d = nc.dram_tensor("y", [BPC, D, L], F32, kind="ExternalOutput").ap()

    with tile.TileContext(nc) as tc:
        with ExitStack() as ctx:
            _body(ctx, tc, nc, xpad_d, dw_d, mdt_d, bw_d, cw_d, prm_d, y_d)
    nc.compile()
    return nc


def _body(ctx, tc, nc, xpad_d, dw_d, mdt_d, bw_d, cw_d, prm_d, y_d):
    const = ctx.enter_context(tc.tile_pool(name="const", bufs=1))
    xpad_pool = ctx.enter_context(tc.tile_pool(name="xpad", bufs=2))
    xs_pool = ctx.enter_context(tc.tile_pool(name="xs", bufs=3))
    big = ctx.enter_context(tc.tile_pool(name="big", bufs=2))
    big1 = ctx.enter_context(tc.tile_pool(name="big1", bufs=2))
    wrk = ctx.enter_context(tc.tile_pool(name="wrk", bufs=1))
    acc_pool = ctx.enter_context(tc.tile_pool(name="acc", bufs=2))
    out_pool = ctx.enter_context(tc.tile_pool(name="out", bufs=2))
    carry_pool = ctx.enter_context(tc.tile_pool(name="carry", bufs=2))
    pconv = ctx.enter_context(tc.tile_pool(name="pconv", bufs=3, space="PSUM"))
    pmm = ctx.enter_context(tc.tile_pool(name="pmm", bufs=1, space="PSUM"))
    pbc = ctx.enter_context(tc.tile_pool(name="pbc", bufs=2, space="PSUM"))

    # ---- load params ----
    dw_t = const.tile([D, 25 * D], F32R)
    nc.sync.dma_start(dw_t[:], dw_d[:].bitcast(F32R))
    mdt_t = const.tile([D, D], F32R)
    nc.sync.dma_start(mdt_t[:], mdt_d[:].bitcast(F32R))
    bw_t = const.tile([D, N * D], F32R)
    nc.sync.dma_start(bw_t[:], bw_d[:].bitcast(F32R))
    cw_t = const.tile([D, N * D], F32R)
    nc.sync.dma_start(cw_t[:], cw_d[:].bitcast(F32R))
    prm_t = const.tile([D, 16], F32)
    nc.sync.dma_start(prm_t[:], prm_d[:])
    convb = prm_t[:, 0:1]
    dtb = prm_t[:, 1:2]
    asc = [prm_t[:, 2 + n:3 + n] for n in range(N)]
    nasc = [prm_t[:, 7 + n:8 + n] for n in range(N)]
    mlnk = prm_t[:, 11:12]
    nconvb = prm_t[:, 12:13]
    ds = prm_t[:, 6:7]

    # PE warmup: ~3.5us of dead matmuls flips the HAM clock gate to 2.4 GHz
    warm_t = const.tile([128, 512], F32)
    nc.gpsimd.memset(warm_t[:], 0.0)
    wps = pconv.tile([128, CHUNK], F32, tag="pc")
    for _ in range(3):
        nc.tensor.matmul(wps[:], lhsT=warm_t[:, :128], rhs=warm_t[:], start=True, stop=True)

    for b in range(BPC):
        carry = carry_pool.tile([D, N + 1], F32)  # col0: cd carry, col 1+n: hs carry

        # ---- conv + SiLU for both halves (batches the Silu ACT-table usage) ----
        xs_halves = []
        for h in range(2):
            row0 = (H // 2) * h
            xpad_t = xpad_pool.tile([D, (H // 2 + 4) * WP], F32R)  # 36 rows x 68
            nc.sync.dma_start(xpad_t[:], xpad_d[b, :, row0 * WP:(row0 + 36) * WP].bitcast(F32R))
            xpv = xpad_t[:].rearrange("p (r c) -> p r c", c=WP)
            xs_t = xs_pool.tile([D, HALF], F32R)
            e_t = wrk.tile([D, HALF], F32, tag="p_t")
            delta_t = big.tile([D, HALF], F32)
            for ci in range(NCH):
                lo, hi = CHUNK * ci, CHUNK * (ci + 1)
                pc = pconv.tile([D, CHUNK], F32)
                r0 = ROWS_PER_CHUNK * ci
                k = 0
                for r in range(KC):
                    for cc in range(KC):
                        rhs = xpv[:, r0 + r:r0 + r + ROWS_PER_CHUNK, cc:cc + W]
                        nc.tensor.matmul(pc[:], lhsT=dw_t[:, k * D:(k + 1) * D], rhs=rhs,
                                         start=(k == 0), stop=(k == 24))
                        k += 1
                # silu without the Silu table set (stays in exp/ln set):
                # sigma = exp(-ln(1 + exp(-(x+b)))); xs = (x+b) * sigma
                s1 = wrk.tile([D, CHUNK], F32, tag="s1")
                nc.scalar.activation(s1[:], pc[:], AFT.Exp, scale=-1.0, bias=nconvb)
                s2 = wrk.tile([D, CHUNK], F32, tag="s2")
                nc.scalar.activation(s2[:], s1[:], AFT.Ln, bias=1.0, scale=1.0)
                sg = wrk.tile([D, CHUNK], F32, tag="sg")
                nc.scalar.activation(sg[:], s2[:], AFT.Exp, scale=-1.0)
                nc.vector.scalar_tensor_tensor(out=xs_t[:, lo:hi], in0=pc[:],
                                               scalar=convb, in1=sg[:],
                                               op0=ALU.add, op1=ALU.mult)
                # delta chunk right behind (PE stream order => early cd start)
                pd = pmm.tile([D, CHUNK], F32)
                nc.tensor.matmul(pd[:], lhsT=mdt_t[:], rhs=xs_t[:, lo:hi],
                                 start=True, stop=True)
                nc.scalar.activation(e_t[:, lo:hi], pd[:], AFT.Exp, bias=dtb, scale=1.0)
                nc.scalar.activation(delta_t[:, lo:hi], e_t[:, lo:hi], AFT.Ln,
                                     bias=1.0, scale=1.0)
            xs_halves.append((xs_t, delta_t))

        for h in range(2):
            xs_t, delta_t = xs_halves[h]

            # ---- cd = cumsum(delta), u = delta * xs ----
            cd_t = big.tile([D, HALF], F32)
            for ci in range(NCH):
                lo, hi = CHUNK * ci, CHUNK * (ci + 1)
                init = (0.0 if h == 0 else carry[:, 0:1]) if ci == 0 else cd_t[:, lo - 1:lo]
                nc.vector.tensor_tensor_scan(cd_t[:, lo:hi], delta_t[:, lo:hi],
                                             delta_t[:, lo:hi], initial=init,
                                             op0=ALU.add, op1=ALU.bypass)
            nc.scalar.copy(carry[:, 0:1], cd_t[:, HALF - 1:HALF])
            u_t = big1.tile([D, HALF], F32)
            nc.vector.tensor_tensor(u_t[:], delta_t[:], xs_t[:], op=ALU.mult)

            yacc = None
            for n in range(N):
                # satP = min(KREC*exp(A_n*cd), 1) = exp(-relu(|A_n|*cd - ln KREC))
                r_t = wrk.tile([D, HALF], F32, tag="p_t")
                nc.scalar.activation(r_t[:], cd_t[:], AFT.Relu, scale=nasc[n], bias=mlnk)
                satp_t = wrk.tile([D, HALF], BF16)
                nc.scalar.activation(satp_t[:], r_t[:], AFT.Exp, scale=-1.0)
                # uB = u * broadcast(Bs_n)
                ub_t = wrk.tile([D, HALF], BF16)
                for ci in range(NCH):
                    pb = pbc.tile([D, CHUNK], F32)
                    nc.tensor.matmul(pb[:], lhsT=bw_t[:, n * D:(n + 1) * D],
                                     rhs=xs_t[:, CHUNK * ci:CHUNK * (ci + 1)],
                                     start=True, stop=True)
                    nc.vector.tensor_tensor(ub_t[:, CHUNK * ci:CHUNK * (ci + 1)],
                                            u_t[:, CHUNK * ci:CHUNK * (ci + 1)],
                                            pb[:], op=ALU.mult)
                d1_t = wrk.tile([D, HALF], BF16)
                nc.vector.tensor_tensor(d1_t[:], ub_t[:], satp_t[:], op=ALU.mult)
                # dA = exp(A_n * delta); hs = scan(dA * state + d1)
                da_t = wrk.tile([D, HALF], F32)
                nc.scalar.activation(da_t[:], delta_t[:], AFT.Exp, scale=asc[n])
                hs_t = wrk.tile([D, HALF], BF16)
                nc.vector.tensor_tensor_scan(hs_t[:], da_t[:], d1_t[:],
                                             initial=(0.0 if h == 0 else carry[:, 1 + n:2 + n]),
                                             op0=ALU.mult, op1=ALU.add)
                nc.scalar.copy(carry[:, 1 + n:2 + n], hs_t[:, HALF - 1:HALF])
                # crep = broadcast(Cs_n) staged to SBUF via ACT
                crep_t = wrk.tile([D, HALF], BF16)
                for ci in range(NCH):
                    pcr = pbc.tile([D, CHUNK], F32)
                    nc.tensor.matmul(pcr[:], lhsT=cw_t[:, n * D:(n + 1) * D],
                                     rhs=xs_t[:, CHUNK * ci:CHUNK * (ci + 1)],
                                     start=True, stop=True)
                    nc.scalar.copy(crep_t[:, CHUNK * ci:CHUNK * (ci + 1)], pcr[:])
                # y accumulation
                if n == 0:
                    yacc = acc_pool.tile([D, HALF], BF16)
                    nc.vector.tensor_tensor(yacc[:], hs_t[:], crep_t[:], op=ALU.mult)
                else:
                    t2_t = wrk.tile([D, HALF], BF16)
                    nc.vector.tensor_tensor(t2_t[:], hs_t[:], crep_t[:], op=ALU.mult)
                    nxt = acc_pool.tile([D, HALF], BF16)
                    nc.vector.tensor_tensor(nxt[:], yacc[:], t2_t[:], op=ALU.add)
                    yacc = nxt

            # ---- y = xs * Ds + y_ssm ---- (chunked so DMA-out overlaps the STT)
            yt = out_pool.tile([D, HALF], F32)
            for ci in range(NCH):
                lo, hi = CHUNK * ci, CHUNK * (ci + 1)
                nc.vector.scalar_tensor_tensor(out=yt[:, lo:hi], in0=xs_t[:, lo:hi],
                                               scalar=ds, in1=yacc[:, lo:hi],
                                               op0=ALU.mult, op1=ALU.add)
                nc.sync.dma_start(y_d[b, :, HALF * h + lo:HALF * h + hi], yt[:, lo:hi])


def _host_params(conv_w, conv_b, x_proj_w, dt_w, dt_b, A_logs, Ds):
    f = np.float32
    dw = np.zeros((D, 25 * D), dtype=f)
    k = 0
    for r in range(KC):
        for cc in range(KC):
            blk = dw[:, k * D:(k + 1) * D]
            np.fill_diagonal(blk, conv_w[:, 0, r, cc])
            k += 1
    M_delta = (dt_w.astype(f) @ x_proj_w[:R].astype(f)).astype(f)   # [D(out), D(in)]
    mdt = np.ascontiguousarray(M_delta.T)                           # lhsT layout [in, out]
    bw = np.zeros((D, N * D), dtype=f)
    cw = np.zeros((D, N * D), dtype=f)
    for n in range(N):
        bw[:, n * D:(n + 1) * D] = x_proj_w[R + n][:, None]
        cw[:, n * D:(n + 1) * D] = x_proj_w[R + N + n][:, None]
    prm = np.zeros((D, 16), dtype=f)
    prm[:, 0] = conv_b
    prm[:, 1] = dt_b
    prm[:, 2:6] = -np.exp(A_logs.astype(f))
    prm[:, 6] = Ds
    prm[:, 7:11] = np.exp(A_logs.astype(f))
    prm[:, 11] = -np.float32(LNKREC)
    prm[:, 12] = -conv_b
    return dw, mdt, bw, cw, prm


_NC_CACHE = {}


def _get_nc():
    if "nc" not in _NC_CACHE:
        _NC_CACHE["nc"] = build_nc()
    return _NC_CACHE["nc"]


def run_on_hw(inputs, trace=False, tmpdir=None):
    """Returns (y_full [32,96,64,64] f32, BassKernelResults)."""
    x = np.asarray(inputs["x"], dtype=np.float32)
    dw, mdt, bw, cw, prm = _host_params(
        np.asarray(inputs["conv_w"], np.float32), np.asarray(inputs["conv_b"], np.float32),
        np.asarray(inputs["x_proj_w"], np.float32), np.asarray(inputs["dt_w"], np.float32),
        np.asarray(inputs["dt_b"], np.float32), np.asarray(inputs["A_logs"], np.float32),
        np.asarray(inputs["Ds"], np.float32))
    xpad = np.zeros((B, D, HP, WP), dtype=np.float32)
    xpad[:, :, 2:-2, 2:-2] = x
    xpad = xpad.reshape(B, D, HP * WP)

    nc = _get_nc()
    in_maps = []
    for i in range(NCORES):
        in_maps.append({
            "xpad": np.ascontiguousarray(xpad[i * BPC:(i + 1) * BPC]),
            "dw": dw, "mdt": mdt, "bw": bw, "cw": cw, "prm": prm,
        })
    res = bass_utils.run_bass_kernel_spmd(nc, in_maps, core_ids=list(range(NCORES)),
                                          trace=trace, tmpdir=tmpdir)
    y = np.concatenate([res.results[i]["y"] for i in range(NCORES)], axis=0)
    return y.reshape(B, D, H, W).astype(np.float32), res


def kernel(**inputs):
    y, _ = run_on_hw(inputs, trace=False)
    return y



# revision 29
# speedup vs baseline: 1.0295x; 1.0295x over previous
"""Trainium2 Bass kernel for CloudSSM: depthwise 5x5 conv + SiLU + selective-scan SSM.

Contract: kernel(**inputs) takes the FULL unsharded inputs (as in setup_inputs())
and returns the FULL [32, 96, 64, 64] float32 output. Internally shards the batch
(32) across 8 NeuronCores (4 per core), runs a Bass/Tile kernel via
run_bass_kernel_spmd, and gathers.

Self-contained: hardcodes all shapes; only imports the concourse runtime.
"""
import sys
import numpy as np
from contextlib import ExitStack

for _p in ("/opt/trn_rl_repo", "/root/.axon_site", "/root/.axon_site/_ro/trn_rl_repo"):
    if _p not in sys.path:
        sys.path.append(_p)

import concourse.bass as bass
import concourse.bacc as bacc
import concourse.tile as tile
from concourse import mybir
from concourse import bass_utils

AFT = mybir.ActivationFunctionType
ALU = mybir.AluOpType
F32 = mybir.dt.float32
F32R = mybir.dt.float32r
BF16 = mybir.dt.bfloat16

# problem shapes (hardcoded per spec)
B, D, H, W = 32, 96, 64, 64
N, R, KC = 4, 6, 5
L = H * W                      # 4096
NCORES = 8
BPC = B // NCORES              # 4 batches per core
HP, WP = H + 4, W + 4          # padded image 68x68
HALF = L // 2                  # 2048
CHUNK = 512
NCH = HALF // CHUNK            # 4 chunks per half
ROWS_PER_CHUNK = CHUNK // W    # 8 image rows per chunk
EPS = np.float32(1e-9)
KREC = float(np.float32(1.0) / EPS)
LNKREC = float(np.log(np.float32(KREC), dtype=np.float32))

CONV_F32R = True               # conv matmuls in fp32r (1 cyc/row) vs fp32 (4 cyc/row)
PROJ_F32R = True               # B/C broadcast matmuls in fp32r


def _mmdt(ap):
    """dtype view used for the fast matmuls"""
    return ap.bitcast(F32R)


def _patch_act_tables():
    """Force all activations onto the natural_log_exp_and_others table set
    (contains Copy/Exp/Identity/Ln/Relu - everything this kernel uses), so the
    whole kernel needs exactly one ACT_TABLE_LOAD instead of thrashing sets."""
    import concourse.hw_specs as hw_specs
    real = hw_specs.get_activation_tables

    def only_lnexp(module_arch):
        t = real(module_arch)
        return {k: (v if k == "natural_log_exp_and_others" else set())
                for k, v in t.items()}

    bacc.get_activation_tables = only_lnexp


def build_nc():
    _patch_act_tables()
    nc = bacc.Bacc("TRN2", target_bir_lowering=False, debug=False)

    xpad_d = nc.dram_tensor("xpad", [BPC, D, HP * WP], F32, kind="ExternalInput").ap()
    dw_d = nc.dram_tensor("dw", [D, 25 * D], F32, kind="ExternalInput").ap()
    mdt_d = nc.dram_tensor("mdt", [D, D], F32, kind="ExternalInput").ap()
    bw_d = nc.dram_tensor("bw", [D, N * D], F32, kind="ExternalInput").ap()
    cw_d = nc.dram_tensor("cw", [D, N * D], F32, kind="ExternalInput").ap()
    # small per-partition params packed in one tensor:
    # col 0: conv_b, col 1: dt_b, cols 2..5: A (=-exp(A_logs)), col 6: Ds
    prm_d = nc.dram_tensor("prm", [D, 16], F32, kind="ExternalInput").ap()
    y_d = nc.dram_tensor("y", [BPC, D, L], F32, kind="ExternalOutput").ap()

    with tile.TileContext(nc) as tc:
        with ExitStack() as ctx:
            _body(ctx, tc, nc, xpad_d, dw_d, mdt_d, bw_d, cw_d, prm_d, y_d)
    nc.compile()
    return nc


def _body(ctx, tc, nc, xpad_d, dw_d, mdt_d, bw_d, cw_d, prm_d, y_d):
    const = ctx.enter_context(tc.tile_pool(name="const", bufs=1))
    xpad_pool = ctx.enter_context(tc.tile_pool(name="xpad", bufs=2))
    xs_pool = ctx.enter_context(tc.tile_pool(name="xs", bufs=3))
    big = ctx.enter_context(tc.tile_pool(name="big", bufs=2))
    big1 = ctx.enter_context(tc.tile_pool(name="big1", bufs=1))
    wrk = ctx.enter_context(tc.tile_pool(name="wrk", bufs=1))
    acc_pool = ctx.enter_context(tc.tile_pool(name="acc", bufs=2))
    out_pool = ctx.enter_context(tc.tile_pool(name="out", bufs=1))
    carry_pool = ctx.enter_context(tc.tile_pool(name="carry", bufs=2))
    pconv = ctx.enter_context(tc.tile_pool(name="pconv", bufs=3, space="PSUM"))
    pmm = ctx.enter_context(tc.tile_pool(name="pmm", bufs=1, space="PSUM"))
    pbc = ctx.enter_context(tc.tile_pool(name="pbc", bufs=2, space="PSUM"))

    # ---- load params ----
    dw_t = const.tile([D, 25 * D], F32R)
    nc.sync.dma_start(dw_t[:], dw_d[:].bitcast(F32R))
    mdt_t = const.tile([D, D], F32R)
    nc.sync.dma_start(mdt_t[:], mdt_d[:].bitcast(F32R))
    bw_t = const.tile([D, N * D], F32R)
    nc.sync.dma_start(bw_t[:], bw_d[:].bitcast(F32R))
    cw_t = const.tile([D, N * D], F32R)
    nc.sync.dma_start(cw_t[:], cw_d[:].bitcast(F32R))
    prm_t = const.tile([D, 16], F32)
    nc.sync.dma_start(prm_t[:], prm_d[:])
    convb = prm_t[:, 0:1]
    dtb = prm_t[:, 1:2]
    asc = [prm_t[:, 2 + n:3 + n] for n in range(N)]
    nasc = [prm_t[:, 7 + n:8 + n] for n in range(N)]
    mlnk = prm_t[:, 11:12]
    nconvb = prm_t[:, 12:13]
    ds = prm_t[:, 6:7]

    # PE warmup: ~3.5us of dead matmuls flips the HAM clock gate to 2.4 GHz
    warm_t = const.tile([128, 512], F32)
    nc.gpsimd.memset(warm_t[:], 0.0)
    wps = pconv.tile([128, CHUNK], F32, tag="pc")
    for _ in range(3):
        nc.tensor.matmul(wps[:], lhsT=warm_t[:, :128], rhs=warm_t[:], start=True, stop=True)

    for b in range(BPC):
        carry = carry_pool.tile([D, N + 1], F32)  # col0: cd carry, col 1+n: hs carry

        # ---- conv + SiLU for both halves (batches the Silu ACT-table usage) ----
        xs_halves = []
        for h in range(2):
            row0 = (H // 2) * h
            xpad_t = xpad_pool.tile([D, (H // 2 + 4) * WP], F32R)  # 36 rows x 68
            nc.sync.dma_start(xpad_t[:], xpad_d[b, :, row0 * WP:(row0 + 36) * WP].bitcast(F32R))
            xpv = xpad_t[:].rearrange("p (r c) -> p r c", c=WP)
            xs_t = xs_pool.tile([D, HALF], F32R)
            e_t = wrk.tile([D, HALF], F32, tag="p_t")
            delta_t = big.tile([D, HALF], F32)
            for ci in range(NCH):
                lo, hi = CHUNK * ci, CHUNK * (ci + 1)
                pc = pconv.tile([D, CHUNK], F32)
                r0 = ROWS_PER_CHUNK * ci
                k = 0
                for r in range(KC):
                    for cc in range(KC):
                        rhs = xpv[:, r0 + r:r0 + r + ROWS_PER_CHUNK, cc:cc + W]
                        nc.tensor.matmul(pc[:], lhsT=dw_t[:, k * D:(k + 1) * D], rhs=rhs,
                                         start=(k == 0), stop=(k == 24))
                        k += 1
                # silu without the Silu table set (stays in exp/ln set):
                # sigma = exp(-ln(1 + exp(-(x+b)))); xs = (x+b) * sigma
                s1 = wrk.tile([D, CHUNK], F32, tag="s1")
                nc.scalar.activation(s1[:], pc[:], AFT.Exp, scale=-1.0, bias=nconvb)
                s2 = wrk.tile([D, CHUNK], F32, tag="s2")
                nc.scalar.activation(s2[:], s1[:], AFT.Ln, bias=1.0, scale=1.0)
                sg = wrk.tile([D, CHUNK], F32, tag="sg")
                nc.scalar.activation(sg[:], s2[:], AFT.Exp, scale=-1.0)
                nc.vector.scalar_tensor_tensor(out=xs_t[:, lo:hi], in0=pc[:],
                                               scalar=convb, in1=sg[:],
                                               op0=ALU.add, op1=ALU.mult)
                # delta chunk right behind (PE stream order => early cd start)
                pd = pmm.tile([D, CHUNK], F32)
                nc.tensor.matmul(pd[:], lhsT=mdt_t[:], rhs=xs_t[:, lo:hi],
                                 start=True, stop=True)
                nc.scalar.activation(e_t[:, lo:hi], pd[:], AFT.Exp, bias=dtb, scale=1.0)
                nc.scalar.activation(delta_t[:, lo:hi], e_t[:, lo:hi], AFT.Ln,
                                     bias=1.0, scale=1.0)
            xs_halves.append((xs_t, delta_t))

        for h in range(2):
            xs_t, delta_t = xs_halves[h]

            # ---- cd = cumsum(delta), u = delta * xs ----
            cd_t = big.tile([D, HALF], F32)
            for ci in range(NCH):
                lo, hi = CHUNK * ci, CHUNK * (ci + 1)
                init = (0.0 if h == 0 else carry[:, 0:1]) if ci == 0 else cd_t[:, lo - 1:lo]
                nc.vector.tensor_tensor_scan(cd_t[:, lo:hi], delta_t[:, lo:hi],
                                             delta_t[:, lo:hi], initial=init,
                                             op0=ALU.add, op1=ALU.bypass)
            nc.scalar.copy(carry[:, 0:1], cd_t[:, HALF - 1:HALF])
            u_t = big1.tile([D, HALF], F32)
            nc.vector.tensor_tensor(u_t[:], delta_t[:], xs_t[:], op=ALU.mult)

            yacc = None
            for n in range(N):
                # satP = min(KREC*exp(A_n*cd), 1) = exp(-relu(|A_n|*cd - ln KREC))
                r_t = wrk.tile([D, HALF], F32, tag="p_t")
                nc.scalar.activation(r_t[:], cd_t[:], AFT.Relu, scale=nasc[n], bias=mlnk)
                satp_t = wrk.tile([D, HALF], BF16)
                nc.scalar.activation(satp_t[:], r_t[:], AFT.Exp, scale=-1.0)
                # uB = u * broadcast(Bs_n)
                ub_t = wrk.tile([D, HALF], BF16)
                for ci in range(NCH):
                    pb = pbc.tile([D, CHUNK], F32)
                    nc.tensor.matmul(pb[:], lhsT=bw_t[:, n * D:(n + 1) * D],
                                     rhs=xs_t[:, CHUNK * ci:CHUNK * (ci + 1)],
                                     start=True, stop=True)
                    nc.vector.tensor_tensor(ub_t[:, CHUNK * ci:CHUNK * (ci + 1)],
                                            u_t[:, CHUNK * ci:CHUNK * (ci + 1)],
                                            pb[:], op=ALU.mult)
                d1_t = wrk.tile([D, HALF], BF16)
                nc.vector.tensor_tensor(d1_t[:], ub_t[:], satp_t[:], op=ALU.mult)
                # dA = exp(A_n * delta); hs = scan(dA * state + d1)
                da_t = wrk.tile([D, HALF], F32)
                nc.scalar.activation(da_t[:], delta_t[:], AFT.Exp, scale=asc[n])
                hs_t = wrk.tile([D, HALF], BF16)
                nc.vector.tensor_tensor_scan(hs_t[:], da_t[:], d1_t[:],
                                             initial=(0.0 if h == 0 else carry[:, 1 + n:2 + n]),
                                             op0=ALU.mult, op1=ALU.add)
                nc.scalar.copy(carry[:, 1 + n:2 + n], hs_t[:, HALF - 1:HALF])
                # crep = broadcast(Cs_n) staged to SBUF via ACT
                crep_t = wrk.tile([D, HALF], BF16)
                for ci in range(NCH):
                    pcr = pbc.tile([D, CHUNK], F32)
                    nc.tensor.matmul(pcr[:], lhsT=cw_t[:, n * D:(n + 1) * D],
                                     rhs=xs_t[:, CHUNK * ci:CHUNK * (ci + 1)],
                                     start=True, stop=True)
                    nc.scalar.copy(crep_t[:, CHUNK * ci:CHUNK * (ci + 1)], pcr[:])
                # y accumulation
                if n == 0:
                    yacc = acc_pool.tile([D, HALF], BF16)
                    nc.vector.tensor_tensor(yacc[:], hs_t[:], crep_t[:], op=ALU.mult)
                else:
                    t2_t = wrk.tile([D, HALF], BF16)
                    nc.vector.tensor_tensor(t2_t[:], hs_t[:], crep_t[:], op=ALU.mult)
                    nxt = acc_pool.tile([D, HALF], BF16)
                    nc.vector.tensor_tensor(nxt[:], yacc[:], t2_t[:], op=ALU.add)
                    yacc = nxt

            # ---- y = xs * Ds + y_ssm ---- (chunked so DMA-out overlaps the STT)
            yt = out_pool.tile([D, HALF], F32)
            for ci in range(NCH):
                lo, hi = CHUNK * ci, CHUNK * (ci + 1)
                nc.vector.scalar_tensor_tensor(out=yt[:, lo:hi], in0=xs_t[:, lo:hi],
                                               scalar=ds, in1=yacc[:, lo:hi],
                                               op0=ALU.mult, op1=ALU.add)
                nc.sync.dma_start(y_d[b, :, HALF * h + lo:HALF * h + hi], yt[:, lo:hi])


def _host_params(conv_w, conv_b, x_proj_w, dt_w, dt_b, A_logs, Ds):
    f = np.float32
    dw = np.zeros((D, 25 * D), dtype=f)
    k = 0
    for r in range(KC):
        for cc in range(KC):
            blk = dw[:, k * D:(k + 1) * D]
            np.fill_diagonal(blk, conv_w[:, 0, r, cc])
            k += 1
    M_delta = (dt_w.astype(f) @ x_proj_w[:R].astype(f)).astype(f)   # [D(out), D(in)]
    mdt = np.ascontiguousarray(M_delta.T)                           # lhsT layout [in, out]
    bw = np.zeros((D, N * D), dtype=f)
    cw = np.zeros((D, N * D), dtype=f)
    for n in range(N):
        bw[:, n * D:(n + 1) * D] = x_proj_w[R + n][:, None]
        cw[:, n * D:(n + 1) * D] = x_proj_w[R + N + n][:, None]
    prm = np.zeros((D, 16), dtype=f)
    prm[:, 0] = conv_b
    prm[:, 1] = dt_b
    prm[:, 2:6] = -np.exp(A_logs.astype(f))
    prm[:, 6] = Ds
    prm[:, 7:11] = np.exp(A_logs.astype(f))
    prm[:, 11] = -np.float32(LNKREC)
    prm[:, 12] = -conv_b
    return dw, mdt, bw, cw, prm


_NC_CACHE = {}


def _get_nc():
    if "nc" not in _NC_CACHE:
        _NC_CACHE["nc"] = build_nc()
    return _NC_CACHE["nc"]


def run_on_hw(inputs, trace=False, tmpdir=None):
    """Returns (y_full [32,96,64,64] f32, BassKernelResults)."""
    x = np.asarray(inputs["x"], dtype=np.float32)
    dw, mdt, bw, cw, prm = _host_params(
        np.asarray(inputs["conv_w"], np.float32), np.asarray(inputs["conv_b"], np.float32),
        np.asarray(inputs["x_proj_w"], np.float32), np.asarray(inputs["dt_w"], np.float32),
        np.asarray(inputs["dt_b"], np.float32), np.asarray(inputs["A_logs"], np.float32),
        np.asarray(inputs["Ds"], np.float32))
    xpad = np.zeros((B, D, HP, WP), dtype=np.float32)
    xpad[:, :, 2:-2, 2:-2] = x
    xpad = xpad.reshape(B, D, HP * WP)

    nc = _get_nc()
    in_maps = []
    for i in range(NCORES):
        in_maps.append({
            "xpad": np.ascontiguousarray(xpad[i * BPC:(i + 1) * BPC]),
            "dw": dw, "mdt": mdt, "bw": bw, "cw": cw, "prm": prm,
        })
    res = bass_utils.run_bass_kernel_spmd(nc, in_maps, core_ids=list(range(NCORES)),
                                          trace=trace, tmpdir=tmpdir)
    y = np.concatenate([res.results[i]["y"] for i in range(NCORES)], axis=0)
    return y.reshape(B, D, H, W).astype(np.float32), res


def kernel(**inputs):
    y, _ = run_on_hw(inputs, trace=False)
    return y

